# revision 9
# baseline (speedup 1.0000x reference)
"""Trainium2 Bass kernel for nn_AttnInteractionLayer_16982300688923.

Math: the reference's einsum 'rfdh,rfoh->rfoh' contracts alpha over its own
softmax axis, which sums to exactly 1 — so the whole Q/K/softmax pipeline
collapses to out == vals.  The remaining computation is

    y   = x @ (m*W_v + (1-m)*W_r)          m = sigmoid(mix)  (host-folded)
    y   = relu(y)
    out = (y - mean(y)) * rsqrt(var(y)+eps) * gamma + beta    (LN over last dim)

Sharding: data-parallel over R across 8 cores; weights replicated.

Per-core device pipeline (rows = R/8 * F = 16384, fp32 throughout):
  DMA x in 1MB slabs -> PE transpose (fp32 has no DMA transpose) ->
  ACT copies X^T PSUM->SBUF -> float32r matmuls accumulate Y[128,512] in PSUM ->
  ACT relu PSUM->SBUF -> DVE bn_stats/bn_aggr -> batched rstd math ->
  GPSIMD applies (y-mu)*rstd -> DMA out.
Uniform gamma/beta fold into the per-row scalars; per-feature gamma/beta get
two extra broadcast passes (general path).
"""

import numpy as np

R, F, D_IN = 2048, 64, 256
OH = 512  # output_dim * num_head
N_CORES = 8
ROWS_PER_CORE = (R // N_CORES) * F  # 16384
P = 128
BLOCKS = ROWS_PER_CORE // P  # 128
SLAB = 8  # 128-row blocks per slab (1MB input DMA)
N_SLABS = BLOCKS // SLAB  # 16
EPS = 1e-5

_prog_cache = {}


def _build(affine_mode, g_u, b_u):
    """affine_mode: 'none' (uniform gamma/beta folded into scalars g_u/b_u)
    or 'full' (per-feature gamma/beta tensors applied on device)."""
    from contextlib import ExitStack

    import concourse.bass as bass
    import concourse.mybir as mybir
    import concourse.tile as tile
    from concourse import bacc
    from concourse.masks import make_identity

    f32 = mybir.dt.float32
    f32r = mybir.dt.float32r
    AF = mybir.ActivationFunctionType
    OP = mybir.AluOpType

    nc = bacc.Bacc(trn_type="TRN2", target_bir_lowering=False)
    xs = nc.dram_tensor("x", [ROWS_PER_CORE, D_IN], f32, kind="ExternalInput")
    wc = nc.dram_tensor("w", [D_IN, OH], f32, kind="ExternalInput")
    if affine_mode == "full":
        gam = nc.dram_tensor("gamma", [OH], f32, kind="ExternalInput")
        bet = nc.dram_tensor("beta", [OH], f32, kind="ExternalInput")
    out = nc.dram_tensor("out", [ROWS_PER_CORE, OH], f32, kind="ExternalOutput")

    x_v = xs.rearrange("(s b p) d -> s p b d", p=P, b=SLAB)
    o_v = out.rearrange("(s b p) n -> s p b n", p=P, b=SLAB)

    with ExitStack() as ctx:
        tc = ctx.enter_context(tile.TileContext(nc))
        const = ctx.enter_context(tc.tile_pool(name="const", bufs=1))
        xin = ctx.enter_context(tc.tile_pool(name="xin", bufs=2))
        lts = ctx.enter_context(tc.tile_pool(name="lts", bufs=3))
        pst = ctx.enter_context(tc.tile_pool(name="pst", bufs=2, space="PSUM"))
        psy = ctx.enter_context(tc.tile_pool(name="psy", bufs=3, space="PSUM"))
        yrp = ctx.enter_context(tc.tile_pool(name="yrp", bufs=2))
        stp = ctx.enter_context(tc.tile_pool(name="stp", bufs=2))
        outp = ctx.enter_context(tc.tile_pool(name="outp", bufs=2))

        w_sb = const.tile([P, 2, OH], f32r)
        nc.sync.dma_start(w_sb, wc.rearrange("(ko p) n -> p ko n", p=P).bitcast(f32r))
        ident = const.tile([P, P], f32)
        make_identity(nc, ident)
        eps_sb = const.tile([P, 1], f32)
        nc.vector.memset(eps_sb, EPS)
        if affine_mode == "full":
            g_sb = const.tile([P, OH], f32)
            b_sb = const.tile([P, OH], f32)
            nc.sync.dma_start(
                g_sb, bass.AP(tensor=gam.tensor, offset=gam.offset, ap=[[0, P], *gam.ap])
            )
            nc.sync.dma_start(
                b_sb, bass.AP(tensor=bet.tensor, offset=bet.offset, ap=[[0, P], *bet.ap])
            )

        for s in range(N_SLABS):
            x_sl = xin.tile([P, SLAB, D_IN], f32)
            nc.sync.dma_start(x_sl, x_v[s])
            yr_sl = yrp.tile([P, SLAB, OH], f32)
            st_sl = stp.tile([P, SLAB, 6], f32)
            mv_sl = stp.tile([P, SLAB, 2], f32)
            rs_sl = stp.tile([P, SLAB], f32)
            nm_sl = stp.tile([P, SLAB], f32)
            out_sl = outp.tile([P, SLAB, OH], f32)

            for b in range(SLAB):
                pt = pst.tile([P, 2, P], f32)
                nc.tensor.transpose(pt[:, 0], x_sl[:, b, 0:P], ident)
                nc.tensor.transpose(pt[:, 1], x_sl[:, b, P : 2 * P], ident)
                lt = lts.tile([P, 2, P], f32r)
                nc.scalar.copy(lt, pt)
                py = psy.tile([P, OH], f32)
                nc.tensor.matmul(
                    py,
                    lt[:, 0],
                    w_sb[:, 0],
                    start=True,
                    stop=False,
                )
                nc.tensor.matmul(
                    py,
                    lt[:, 1],
                    w_sb[:, 1],
                    start=False,
                    stop=True,
                )
                nc.scalar.activation(yr_sl[:, b], py, AF.Relu)
                nc.vector.bn_stats(st_sl[:, b], yr_sl[:, b])
                nc.vector.bn_aggr(mv_sl[:, b], st_sl[:, b])

            # Per-slab LN scalar math on [P, SLAB] tiles:
            #   rstd = 1/sqrt(var+eps) (*gamma_u);  nm = -mu*rstd (*gamma_u + beta_u)
            nc.scalar.activation(rs_sl, mv_sl[:, :, 1], AF.Sqrt, bias=eps_sb)
            nc.vector.reciprocal(rs_sl, rs_sl)
            nc.vector.tensor_scalar_mul(nm_sl, mv_sl[:, :, 0], -1.0)
            nc.vector.tensor_tensor(nm_sl, nm_sl, rs_sl, OP.mult)
            if affine_mode == "none" and (g_u != 1.0 or b_u != 0.0):
                if g_u != 1.0:
                    nc.vector.tensor_scalar_mul(rs_sl, rs_sl, float(g_u))
                nc.vector.tensor_scalar(
                    nm_sl, nm_sl, float(g_u), float(b_u), OP.mult, OP.add
                )

            for b in range(SLAB):
                if affine_mode == "full":
                    nc.gpsimd.tensor_scalar(
                        out_sl[:, b],
                        yr_sl[:, b],
                        rs_sl[:, b : b + 1],
                        nm_sl[:, b : b + 1],
                        OP.mult,
                        OP.add,
                    )
                    nc.vector.tensor_tensor(out_sl[:, b], out_sl[:, b], g_sb, OP.mult)
                    nc.vector.tensor_tensor(out_sl[:, b], out_sl[:, b], b_sb, OP.add)
                else:
                    nc.gpsimd.tensor_scalar(
                        out_sl[:, b],
                        yr_sl[:, b],
                        rs_sl[:, b : b + 1],
                        nm_sl[:, b : b + 1],
                        OP.mult,
                        OP.add,
                    )
            nc.sync.dma_start(o_v[s], out_sl)
    nc.finalize()
    return nc


def _get_prog(affine_mode, g_u, b_u):
    key = (affine_mode, g_u, b_u)
    if key not in _prog_cache:
        _prog_cache[key] = _build(affine_mode, g_u, b_u)
    return _prog_cache[key]


def _prepare(x, W_q, W_k, W_v, W_r, mix, gamma, beta):
    x = np.ascontiguousarray(np.asarray(x, dtype=np.float32))
    W_v = np.asarray(W_v, dtype=np.float32)
    W_r = np.asarray(W_r, dtype=np.float32)
    gamma = np.asarray(gamma, dtype=np.float32)
    beta = np.asarray(beta, dtype=np.float32)
    m = 1.0 / (1.0 + np.exp(-float(np.asarray(mix).reshape(-1)[0])))
    wc = np.ascontiguousarray((m * W_v + (1.0 - m) * W_r).astype(np.float32))

    if np.all(gamma == gamma.flat[0]) and np.all(beta == beta.flat[0]):
        affine_mode, g_u, b_u = "none", float(gamma.flat[0]), float(beta.flat[0])
    else:
        affine_mode, g_u, b_u = "full", 1.0, 0.0

    x_flat = x.reshape(R * F, D_IN)
    in_maps = []
    for c in range(N_CORES):
        im = {
            "x": np.ascontiguousarray(x_flat[c * ROWS_PER_CORE : (c + 1) * ROWS_PER_CORE]),
            "w": wc,
        }
        if affine_mode == "full":
            im["gamma"] = gamma
            im["beta"] = beta
        in_maps.append(im)
    return in_maps, affine_mode, g_u, b_u


def run(trace=False, **inputs):
    """Internal entry: returns (output, BassKernelResults)."""
    from concourse.bass_utils import run_bass_kernel_spmd

    in_maps, affine_mode, g_u, b_u = _prepare(**inputs)
    nc = _get_prog(affine_mode, g_u, b_u)
    res = run_bass_kernel_spmd(nc, in_maps, core_ids=list(range(N_CORES)), trace=trace)
    parts = [r["out"].reshape(R // N_CORES, F, OH) for r in res.results]
    return np.concatenate(parts, axis=0), res


def kernel(**inputs):
    out, _ = run(trace=False, **inputs)
    return out


# revision 16
# speedup vs baseline: 1.0210x; 1.0210x over previous
"""Trainium2 Bass kernel for nn_AttnInteractionLayer_16982300688923.

Math: the reference's einsum 'rfdh,rfoh->rfoh' contracts alpha over its own
softmax axis, which sums to exactly 1 — so the whole Q/K/softmax pipeline
collapses to out == vals.  The remaining computation is

    y   = x @ (m*W_v + (1-m)*W_r)          m = sigmoid(mix)  (host-folded)
    y   = relu(y)
    out = (y - mean(y)) * rsqrt(var(y)+eps) * gamma + beta    (LN over last dim)

Sharding: data-parallel over R across 8 cores; weights replicated.  X is
pre-transposed on the host while sharding so the contraction dim lands on
SBUF partitions with fast contiguous DMAs (fp32 has no DMA-transpose path).

Per-core device pipeline (rows = R/8 * F = 16384, fp32):
  DMA x^T slabs -> float32r matmuls accumulate Y[128,512] in PSUM ->
  ACT relu PSUM->SBUF (+sum accum) -> DVE square (+sum-of-squares accum) ->
  batched LN scalar math -> apply (y-mu)*rstd split across GPSIMD/ACT/DVE ->
  DMA out.
Uniform gamma/beta fold into the per-row scalars; per-feature gamma/beta get
two extra broadcast passes (general path).
"""

import numpy as np

R, F, D_IN = 2048, 64, 256
OH = 512  # output_dim * num_head
N_CORES = 8
ROWS_PER_CORE = (R // N_CORES) * F  # 16384
P = 128
BLOCKS = ROWS_PER_CORE // P  # 128
SLAB = 8  # 128-row blocks per slab
N_SLABS = BLOCKS // SLAB  # 16
EPS = 1e-5

# apply-pass engine per block-in-slab: balance GPSIMD / ACT / DVE
import os as _os

APPLY_ENGINES = list(_os.environ.get("K_APPLY", "GGGAAAVV"))
STATS_MODE = _os.environ.get("K_STATS", "accum")  # accum | bn

_prog_cache = {}


def _build(affine_mode, g_u, b_u):
    """affine_mode: 'none' (uniform gamma/beta folded into scalars g_u/b_u)
    or 'full' (per-feature gamma/beta tensors applied on device)."""
    from contextlib import ExitStack

    import concourse.bass as bass
    import concourse.mybir as mybir
    import concourse.tile as tile
    from concourse import bacc

    f32 = mybir.dt.float32
    f32r = mybir.dt.float32r
    AF = mybir.ActivationFunctionType
    OP = mybir.AluOpType

    nc = bacc.Bacc(trn_type="TRN2", target_bir_lowering=False)
    xt = nc.dram_tensor("xt", [D_IN, ROWS_PER_CORE], f32, kind="ExternalInput")
    wc = nc.dram_tensor("w", [D_IN, OH], f32, kind="ExternalInput")
    if affine_mode == "full":
        gam = nc.dram_tensor("gamma", [OH], f32, kind="ExternalInput")
        bet = nc.dram_tensor("beta", [OH], f32, kind="ExternalInput")
    out = nc.dram_tensor("out", [ROWS_PER_CORE, OH], f32, kind="ExternalOutput")

    # [128 part, ko, rows]; contiguous 4KB runs per (p, ko) per slab
    xt_v = xt.rearrange("(ko p) r -> p ko r", p=P).bitcast(f32r)
    o_v = out.rearrange("(s b p) n -> s p b n", p=P, b=SLAB)

    with ExitStack() as ctx:
        tc = ctx.enter_context(tile.TileContext(nc))
        const = ctx.enter_context(tc.tile_pool(name="const", bufs=1))
        xin = ctx.enter_context(tc.tile_pool(name="xin", bufs=2))
        psy = ctx.enter_context(tc.tile_pool(name="psy", bufs=4, space="PSUM"))
        yrp = ctx.enter_context(tc.tile_pool(name="yrp", bufs=2))
        sqp = ctx.enter_context(tc.tile_pool(name="sqp", bufs=3))
        stp = ctx.enter_context(tc.tile_pool(name="stp", bufs=2))
        outp = ctx.enter_context(tc.tile_pool(name="outp", bufs=2))

        w_sb = const.tile([P, 2, OH], f32r)
        nc.sync.dma_start(w_sb, wc.rearrange("(ko p) n -> p ko n", p=P).bitcast(f32r))
        eps_sb = const.tile([P, 1], f32)
        nc.vector.memset(eps_sb, EPS)
        if affine_mode == "full":
            g_sb = const.tile([P, OH], f32)
            b_sb = const.tile([P, OH], f32)
            nc.sync.dma_start(
                g_sb, bass.AP(tensor=gam.tensor, offset=gam.offset, ap=[[0, P], *gam.ap])
            )
            nc.sync.dma_start(
                b_sb, bass.AP(tensor=bet.tensor, offset=bet.offset, ap=[[0, P], *bet.ap])
            )

        for s in range(N_SLABS):
            xt_sl = xin.tile([P, 2, SLAB * P], f32r)
            nc.sync.dma_start(xt_sl, xt_v[:, :, s * SLAB * P : (s + 1) * SLAB * P])
            yr_sl = yrp.tile([P, SLAB, OH], f32)
            sum_sl = stp.tile([P, SLAB], f32)
            ssq_sl = stp.tile([P, SLAB], f32)
            mv_sl = stp.tile([P, SLAB, 2], f32)
            negmu_sl = stp.tile([P, SLAB], f32)
            musq_sl = stp.tile([P, SLAB], f32)
            var_sl = stp.tile([P, SLAB], f32)
            rs_sl = stp.tile([P, SLAB], f32)
            nm_sl = stp.tile([P, SLAB], f32)
            out_sl = outp.tile([P, SLAB, OH], f32)

            for b in range(SLAB):
                py = psy.tile([P, OH], f32)
                nc.tensor.matmul(
                    py, xt_sl[:, 0, b * P : (b + 1) * P], w_sb[:, 0],
                    start=True, stop=False,
                )
                nc.tensor.matmul(
                    py, xt_sl[:, 1, b * P : (b + 1) * P], w_sb[:, 1],
                    start=False, stop=True,
                )
                if STATS_MODE == "accum":
                    nc.scalar.activation(
                        yr_sl[:, b], py, AF.Relu, accum_out=sum_sl[:, b : b + 1]
                    )
                    sq = sqp.tile([P, 1], f32)
                    nc.vector.tensor_tensor_reduce(
                        out=sq.broadcast_to(yr_sl[:, b].shape),
                        in0=yr_sl[:, b],
                        in1=yr_sl[:, b],
                        scale=1.0,
                        scalar=0.0,
                        op0=OP.mult,
                        op1=OP.add,
                        accum_out=ssq_sl[:, b : b + 1],
                    )
                elif STATS_MODE == "bnacc":
                    # bn stats path but with accum_out on the relu (bisect probe)
                    nc.scalar.activation(
                        yr_sl[:, b], py, AF.Relu, accum_out=sum_sl[:, b : b + 1]
                    )
                    st = sqp.tile([P, 6], f32, tag="bnst")
                    nc.vector.bn_stats(st, yr_sl[:, b])
                    nc.vector.bn_aggr(mv_sl[:, b], st)
                else:
                    nc.scalar.activation(yr_sl[:, b], py, AF.Relu)
                    st = sqp.tile([P, 6], f32, tag="bnst")
                    nc.vector.bn_stats(st, yr_sl[:, b])
                    nc.vector.bn_aggr(mv_sl[:, b], st)

            # Per-slab LN scalar math on [P, SLAB] tiles:
            #   mu = sum/OH; var = ssq/OH - mu^2; rstd = 1/sqrt(var+eps)
            #   out = y*rstd + (-mu*rstd)    (gamma_u/beta_u folded in)
            if STATS_MODE == "accum":
                nc.vector.tensor_scalar_mul(negmu_sl, sum_sl, -1.0 / OH)
                nc.vector.tensor_tensor(musq_sl, negmu_sl, negmu_sl, OP.mult)
                nc.scalar.mul(var_sl, ssq_sl, 1.0 / OH)
                nc.vector.tensor_tensor(var_sl, var_sl, musq_sl, OP.subtract)
            else:
                nc.vector.tensor_scalar_mul(negmu_sl, mv_sl[:, :, 0], -1.0)
                nc.vector.tensor_copy(var_sl, mv_sl[:, :, 1])
                if STATS_MODE == "bnacc":
                    # consume sum_sl so it isn't dead code
                    nc.vector.tensor_scalar_mul(musq_sl, sum_sl, 0.0)
            nc.scalar.activation(rs_sl, var_sl, AF.Sqrt, bias=eps_sb)
            nc.vector.reciprocal(rs_sl, rs_sl)
            nc.vector.tensor_tensor(nm_sl, negmu_sl, rs_sl, OP.mult)
            if affine_mode == "none" and (g_u != 1.0 or b_u != 0.0):
                if g_u != 1.0:
                    nc.vector.tensor_scalar_mul(rs_sl, rs_sl, float(g_u))
                nc.vector.tensor_scalar(
                    nm_sl, nm_sl, float(g_u), float(b_u), OP.mult, OP.add
                )

            for b in range(SLAB):
                rs_ap = rs_sl[:, b : b + 1]
                nm_ap = nm_sl[:, b : b + 1]
                eng = APPLY_ENGINES[b]
                if eng == "A":
                    nc.scalar.activation(
                        out_sl[:, b], yr_sl[:, b], AF.Identity, bias=nm_ap, scale=rs_ap
                    )
                elif eng == "V":
                    nc.vector.tensor_scalar(
                        out_sl[:, b], yr_sl[:, b], rs_ap, nm_ap, OP.mult, OP.add
                    )
                else:
                    nc.gpsimd.tensor_scalar(
                        out_sl[:, b], yr_sl[:, b], rs_ap, nm_ap, OP.mult, OP.add
                    )
                if affine_mode == "full":
                    nc.vector.tensor_tensor(out_sl[:, b], out_sl[:, b], g_sb, OP.mult)
                    nc.gpsimd.tensor_tensor(out_sl[:, b], out_sl[:, b], b_sb, OP.add)
            nc.sync.dma_start(o_v[s], out_sl)
    nc.finalize()
    return nc


def _get_prog(affine_mode, g_u, b_u):
    key = (affine_mode, g_u, b_u)
    if key not in _prog_cache:
        _prog_cache[key] = _build(affine_mode, g_u, b_u)
    return _prog_cache[key]


def _prepare(x, W_q, W_k, W_v, W_r, mix, gamma, beta):
    x = np.asarray(x, dtype=np.float32)
    W_v = np.asarray(W_v, dtype=np.float32)
    W_r = np.asarray(W_r, dtype=np.float32)
    gamma = np.asarray(gamma, dtype=np.float32)
    beta = np.asarray(beta, dtype=np.float32)
    m = 1.0 / (1.0 + np.exp(-float(np.asarray(mix).reshape(-1)[0])))
    wc = np.ascontiguousarray((m * W_v + (1.0 - m) * W_r).astype(np.float32))

    if np.all(gamma == gamma.flat[0]) and np.all(beta == beta.flat[0]):
        affine_mode, g_u, b_u = "none", float(gamma.flat[0]), float(beta.flat[0])
    else:
        affine_mode, g_u, b_u = "full", 1.0, 0.0

    x_flat = x.reshape(R * F, D_IN)
    in_maps = []
    for c in range(N_CORES):
        shard = x_flat[c * ROWS_PER_CORE : (c + 1) * ROWS_PER_CORE]
        im = {
            "xt": np.ascontiguousarray(shard.T),
            "w": wc,
        }
        if affine_mode == "full":
            im["gamma"] = gamma
            im["beta"] = beta
        in_maps.append(im)
    return in_maps, affine_mode, g_u, b_u


def run(trace=False, **inputs):
    """Internal entry: returns (output, BassKernelResults)."""
    from concourse.bass_utils import run_bass_kernel_spmd

    in_maps, affine_mode, g_u, b_u = _prepare(**inputs)
    nc = _get_prog(affine_mode, g_u, b_u)
    res = run_bass_kernel_spmd(nc, in_maps, core_ids=list(range(N_CORES)), trace=trace)
    parts = [r["out"].reshape(R // N_CORES, F, OH) for r in res.results]
    return np.concatenate(parts, axis=0), res


def kernel(**inputs):
    out, _ = run(trace=False, **inputs)
    return out


# revision 19
# speedup vs baseline: 1.1448x; 1.1212x over previous
"""Trainium2 Bass kernel for nn_AttnInteractionLayer_16982300688923.

Math: the reference's einsum 'rfdh,rfoh->rfoh' contracts alpha over its own
softmax axis, which sums to exactly 1 — so the whole Q/K/softmax pipeline
collapses to out == vals.  The remaining computation is

    y   = x @ (m*W_v + (1-m)*W_r)          m = sigmoid(mix)  (host-folded)
    y   = relu(y)
    out = (y - mean(y)) * rsqrt(var(y)+eps) * gamma + beta    (LN over last dim)

Sharding: data-parallel over R across 8 cores; weights replicated.  X is
pre-transposed on the host while sharding so the contraction dim lands on
SBUF partitions with fast contiguous DMAs (fp32 has no DMA-transpose path).

Per-core device pipeline (rows = R/8 * F = 16384, fp32):
  DMA x^T slabs -> float32r matmuls accumulate Y[128,512] in PSUM ->
  ACT relu PSUM->SBUF (+sum accum) -> DVE square (+sum-of-squares accum) ->
  batched LN scalar math -> apply (y-mu)*rstd split across GPSIMD/ACT/DVE ->
  DMA out.
Uniform gamma/beta fold into the per-row scalars; per-feature gamma/beta get
two extra broadcast passes (general path).
"""

import numpy as np

R, F, D_IN = 2048, 64, 256
OH = 512  # output_dim * num_head
N_CORES = 8
ROWS_PER_CORE = (R // N_CORES) * F  # 16384
P = 128
BLOCKS = ROWS_PER_CORE // P  # 128
SLAB = 8  # 128-row blocks per slab
N_SLABS = BLOCKS // SLAB  # 16
EPS = 1e-5

# apply-pass engine per block-in-slab: balance GPSIMD / ACT / DVE
import os as _os

APPLY_ENGINES = list(_os.environ.get("K_APPLY", "GGGGGGVV"))
STATS_MODE = _os.environ.get("K_STATS", "bn")  # accum | bnacc | bn

_prog_cache = {}


def _build(affine_mode, g_u, b_u):
    """affine_mode: 'none' (uniform gamma/beta folded into scalars g_u/b_u)
    or 'full' (per-feature gamma/beta tensors applied on device)."""
    from contextlib import ExitStack

    import concourse.bass as bass
    import concourse.mybir as mybir
    import concourse.tile as tile
    from concourse import bacc

    f32 = mybir.dt.float32
    f32r = mybir.dt.float32r
    AF = mybir.ActivationFunctionType
    OP = mybir.AluOpType

    nc = bacc.Bacc(trn_type="TRN2", target_bir_lowering=False)
    xt = nc.dram_tensor("xt", [D_IN, ROWS_PER_CORE], f32, kind="ExternalInput")
    wc = nc.dram_tensor("w", [D_IN, OH], f32, kind="ExternalInput")
    if affine_mode == "full":
        gam = nc.dram_tensor("gamma", [OH], f32, kind="ExternalInput")
        bet = nc.dram_tensor("beta", [OH], f32, kind="ExternalInput")
    out = nc.dram_tensor("out", [ROWS_PER_CORE, OH], f32, kind="ExternalOutput")

    # [128 part, ko, rows]; contiguous 4KB runs per (p, ko) per slab
    xt_v = xt.rearrange("(ko p) r -> p ko r", p=P).bitcast(f32r)
    o_v = out.rearrange("(s b p) n -> s p b n", p=P, b=SLAB)

    with ExitStack() as ctx:
        tc = ctx.enter_context(tile.TileContext(nc))
        const = ctx.enter_context(tc.tile_pool(name="const", bufs=1))
        xin = ctx.enter_context(tc.tile_pool(name="xin", bufs=3))
        psy = ctx.enter_context(tc.tile_pool(name="psy", bufs=6, space="PSUM"))
        yrp = ctx.enter_context(tc.tile_pool(name="yrp", bufs=3))
        sqp = ctx.enter_context(tc.tile_pool(name="sqp", bufs=4))
        stp = ctx.enter_context(tc.tile_pool(name="stp", bufs=3))
        outp = ctx.enter_context(tc.tile_pool(name="outp", bufs=3))

        w_sb = const.tile([P, 2, OH], f32r)
        nc.sync.dma_start(w_sb, wc.rearrange("(ko p) n -> p ko n", p=P).bitcast(f32r))
        eps_sb = const.tile([P, 1], f32)
        nc.vector.memset(eps_sb, EPS)
        if affine_mode == "full":
            g_sb = const.tile([P, OH], f32)
            b_sb = const.tile([P, OH], f32)
            nc.sync.dma_start(
                g_sb, bass.AP(tensor=gam.tensor, offset=gam.offset, ap=[[0, P], *gam.ap])
            )
            nc.sync.dma_start(
                b_sb, bass.AP(tensor=bet.tensor, offset=bet.offset, ap=[[0, P], *bet.ap])
            )

        for s in range(N_SLABS):
            xt_sl = xin.tile([P, 2, SLAB * P], f32r)
            nc.sync.dma_start(xt_sl, xt_v[:, :, s * SLAB * P : (s + 1) * SLAB * P])
            yr_sl = yrp.tile([P, SLAB, OH], f32)
            sum_sl = stp.tile([P, SLAB], f32)
            ssq_sl = stp.tile([P, SLAB], f32)
            mv_sl = stp.tile([P, SLAB, 2], f32)
            negmu_sl = stp.tile([P, SLAB], f32)
            musq_sl = stp.tile([P, SLAB], f32)
            var_sl = stp.tile([P, SLAB], f32)
            rs_sl = stp.tile([P, SLAB], f32)
            nm_sl = stp.tile([P, SLAB], f32)
            out_sl = outp.tile([P, SLAB, OH], f32)

            for b in range(SLAB):
                py = psy.tile([P, OH], f32)
                nc.tensor.matmul(
                    py, xt_sl[:, 0, b * P : (b + 1) * P], w_sb[:, 0],
                    start=True, stop=False,
                )
                nc.tensor.matmul(
                    py, xt_sl[:, 1, b * P : (b + 1) * P], w_sb[:, 1],
                    start=False, stop=True,
                )
                if STATS_MODE == "accum":
                    nc.scalar.activation(
                        yr_sl[:, b], py, AF.Relu, accum_out=sum_sl[:, b : b + 1]
                    )
                    sq = sqp.tile([P, 1], f32)
                    nc.vector.tensor_tensor_reduce(
                        out=sq.broadcast_to(yr_sl[:, b].shape),
                        in0=yr_sl[:, b],
                        in1=yr_sl[:, b],
                        scale=1.0,
                        scalar=0.0,
                        op0=OP.mult,
                        op1=OP.add,
                        accum_out=ssq_sl[:, b : b + 1],
                    )
                elif STATS_MODE == "bnacc":
                    # bn stats path but with accum_out on the relu (bisect probe)
                    nc.scalar.activation(
                        yr_sl[:, b], py, AF.Relu, accum_out=sum_sl[:, b : b + 1]
                    )
                    st = sqp.tile([P, 6], f32, tag="bnst")
                    nc.vector.bn_stats(st, yr_sl[:, b])
                    nc.vector.bn_aggr(mv_sl[:, b], st)
                else:
                    nc.scalar.activation(yr_sl[:, b], py, AF.Relu)
                    st = sqp.tile([P, 6], f32, tag="bnst")
                    nc.vector.bn_stats(st, yr_sl[:, b])
                    nc.vector.bn_aggr(mv_sl[:, b], st)

            # Per-slab LN scalar math on [P, SLAB] tiles:
            #   mu = sum/OH; var = ssq/OH - mu^2; rstd = 1/sqrt(var+eps)
            #   out = y*rstd + (-mu*rstd)    (gamma_u/beta_u folded in)
            if STATS_MODE == "accum":
                nc.vector.tensor_scalar_mul(negmu_sl, sum_sl, -1.0 / OH)
                nc.vector.tensor_tensor(musq_sl, negmu_sl, negmu_sl, OP.mult)
                nc.scalar.mul(var_sl, ssq_sl, 1.0 / OH)
                nc.vector.tensor_tensor(var_sl, var_sl, musq_sl, OP.subtract)
            else:
                nc.vector.tensor_scalar_mul(negmu_sl, mv_sl[:, :, 0], -1.0)
                nc.vector.tensor_copy(var_sl, mv_sl[:, :, 1])
                if STATS_MODE == "bnacc":
                    # consume sum_sl so it isn't dead code
                    nc.vector.tensor_scalar_mul(musq_sl, sum_sl, 0.0)
            nc.scalar.activation(rs_sl, var_sl, AF.Sqrt, bias=eps_sb)
            nc.vector.reciprocal(rs_sl, rs_sl)
            nc.vector.tensor_tensor(nm_sl, negmu_sl, rs_sl, OP.mult)
            if affine_mode == "none" and (g_u != 1.0 or b_u != 0.0):
                if g_u != 1.0:
                    nc.vector.tensor_scalar_mul(rs_sl, rs_sl, float(g_u))
                nc.vector.tensor_scalar(
                    nm_sl, nm_sl, float(g_u), float(b_u), OP.mult, OP.add
                )

            for b in range(SLAB):
                rs_ap = rs_sl[:, b : b + 1]
                nm_ap = nm_sl[:, b : b + 1]
                eng = APPLY_ENGINES[b]
                if eng == "A":
                    nc.scalar.activation(
                        out_sl[:, b], yr_sl[:, b], AF.Identity, bias=nm_ap, scale=rs_ap
                    )
                elif eng == "V":
                    nc.vector.tensor_scalar(
                        out_sl[:, b], yr_sl[:, b], rs_ap, nm_ap, OP.mult, OP.add
                    )
                else:
                    nc.gpsimd.tensor_scalar(
                        out_sl[:, b], yr_sl[:, b], rs_ap, nm_ap, OP.mult, OP.add
                    )
                if affine_mode == "full":
                    nc.vector.tensor_tensor(out_sl[:, b], out_sl[:, b], g_sb, OP.mult)
                    nc.gpsimd.tensor_tensor(out_sl[:, b], out_sl[:, b], b_sb, OP.add)
                if b == SLAB // 2 - 1:
                    nc.sync.dma_start(
                        o_v[s, :, : SLAB // 2], out_sl[:, : SLAB // 2]
                    )
            nc.sync.dma_start(o_v[s, :, SLAB // 2 :], out_sl[:, SLAB // 2 :])
    nc.finalize()
    return nc


def _get_prog(affine_mode, g_u, b_u):
    key = (affine_mode, g_u, b_u)
    if key not in _prog_cache:
        _prog_cache[key] = _build(affine_mode, g_u, b_u)
    return _prog_cache[key]


def _prepare(x, W_q, W_k, W_v, W_r, mix, gamma, beta):
    x = np.asarray(x, dtype=np.float32)
    W_v = np.asarray(W_v, dtype=np.float32)
    W_r = np.asarray(W_r, dtype=np.float32)
    gamma = np.asarray(gamma, dtype=np.float32)
    beta = np.asarray(beta, dtype=np.float32)
    m = 1.0 / (1.0 + np.exp(-float(np.asarray(mix).reshape(-1)[0])))
    wc = np.ascontiguousarray((m * W_v + (1.0 - m) * W_r).astype(np.float32))

    if np.all(gamma == gamma.flat[0]) and np.all(beta == beta.flat[0]):
        affine_mode, g_u, b_u = "none", float(gamma.flat[0]), float(beta.flat[0])
    else:
        affine_mode, g_u, b_u = "full", 1.0, 0.0

    x_flat = x.reshape(R * F, D_IN)
    in_maps = []
    for c in range(N_CORES):
        shard = x_flat[c * ROWS_PER_CORE : (c + 1) * ROWS_PER_CORE]
        im = {
            "xt": np.ascontiguousarray(shard.T),
            "w": wc,
        }
        if affine_mode == "full":
            im["gamma"] = gamma
            im["beta"] = beta
        in_maps.append(im)
    return in_maps, affine_mode, g_u, b_u


def run(trace=False, **inputs):
    """Internal entry: returns (output, BassKernelResults)."""
    from concourse.bass_utils import run_bass_kernel_spmd

    in_maps, affine_mode, g_u, b_u = _prepare(**inputs)
    nc = _get_prog(affine_mode, g_u, b_u)
    res = run_bass_kernel_spmd(nc, in_maps, core_ids=list(range(N_CORES)), trace=trace)
    parts = [r["out"].reshape(R // N_CORES, F, OH) for r in res.results]
    return np.concatenate(parts, axis=0), res


def kernel(**inputs):
    out, _ = run(trace=False, **inputs)
    return out


# revision 26
# speedup vs baseline: 1.2715x; 1.1107x over previous
"""Trainium2 Bass kernel for nn_AttnInteractionLayer_16982300688923.

Math: the reference's einsum 'rfdh,rfoh->rfoh' contracts alpha over its own
softmax axis, which sums to exactly 1 — so the whole Q/K/softmax pipeline
collapses to out == vals.  The remaining computation is

    y   = x @ (m*W_v + (1-m)*W_r)          m = sigmoid(mix)  (host-folded)
    y   = relu(y)
    out = (y - mean(y)) * rsqrt(var(y)+eps) * gamma + beta    (LN over last dim)

Sharding: data-parallel over R across 8 cores; weights replicated.  X is
pre-transposed on the host while sharding so the contraction dim lands on
SBUF partitions with fast contiguous DMAs (fp32 has no DMA-transpose path).

Per-core device pipeline (rows = R/8 * F = 16384, fp32):
  DMA x^T slabs -> float32r matmuls accumulate Y[128,512] in PSUM ->
  ACT relu PSUM->SBUF (+sum accum) -> DVE square (+sum-of-squares accum) ->
  batched LN scalar math -> apply (y-mu)*rstd split across GPSIMD/ACT/DVE ->
  DMA out.
Uniform gamma/beta fold into the per-row scalars; per-feature gamma/beta get
two extra broadcast passes (general path).
"""

import numpy as np

R, F, D_IN = 2048, 64, 256
OH = 512  # output_dim * num_head
N_CORES = 8
ROWS_PER_CORE = (R // N_CORES) * F  # 16384
P = 128
BLOCKS = ROWS_PER_CORE // P  # 128
SLAB = 8  # 128-row blocks per slab
N_SLABS = BLOCKS // SLAB  # 16
EPS = 1e-5

# apply-pass engine per block-in-slab: balance GPSIMD / ACT / DVE
import os as _os

APPLY_ENGINES = list(_os.environ.get("K_APPLY", "GGGGGGVV"))
STATS_MODE = _os.environ.get("K_STATS", "bn")  # accum | bnacc | bn

_prog_cache = {}


def _build(affine_mode, g_u, b_u):
    """affine_mode: 'none' (uniform gamma/beta folded into scalars g_u/b_u)
    or 'full' (per-feature gamma/beta tensors applied on device)."""
    from contextlib import ExitStack

    import concourse.bass as bass
    import concourse.mybir as mybir
    import concourse.tile as tile
    from concourse import bacc

    f32 = mybir.dt.float32
    f32r = mybir.dt.float32r
    AF = mybir.ActivationFunctionType
    OP = mybir.AluOpType

    nc = bacc.Bacc(trn_type="TRN2", target_bir_lowering=False)
    # Host-permuted input: [p, s, ko, r] so each partition reads one
    # contiguous 8KB run per slab.
    xt = nc.dram_tensor(
        "xt", [P, N_SLABS, 2, SLAB * P], f32, kind="ExternalInput"
    )
    wc = nc.dram_tensor("w", [D_IN, OH], f32, kind="ExternalInput")
    if affine_mode == "full":
        gam = nc.dram_tensor("gamma", [OH], f32, kind="ExternalInput")
        bet = nc.dram_tensor("beta", [OH], f32, kind="ExternalInput")
    # Host-unpermuted output: [s, p, b, n] so each partition writes one
    # contiguous 16KB run per slab (8KB per half-slab DMA).
    out = nc.dram_tensor(
        "out", [N_SLABS, P, SLAB, OH], f32, kind="ExternalOutput"
    )

    xt_v = xt.bitcast(f32r)
    o_v = out

    with ExitStack() as ctx:
        tc = ctx.enter_context(tile.TileContext(nc))
        const = ctx.enter_context(tc.tile_pool(name="const", bufs=1))
        xin = ctx.enter_context(tc.tile_pool(name="xin", bufs=4))
        psy = ctx.enter_context(tc.tile_pool(name="psy", bufs=6, space="PSUM"))
        yrp = ctx.enter_context(tc.tile_pool(name="yrp", bufs=3))
        sqp = ctx.enter_context(tc.tile_pool(name="sqp", bufs=4))
        stp = ctx.enter_context(tc.tile_pool(name="stp", bufs=4))
        outp = ctx.enter_context(tc.tile_pool(name="outp", bufs=4))

        w_sb = const.tile([P, 2, OH], f32r)
        nc.sync.dma_start(w_sb, wc.rearrange("(ko p) n -> p ko n", p=P).bitcast(f32r))
        eps_sb = const.tile([P, 1], f32)
        nc.vector.memset(eps_sb, EPS)
        if affine_mode == "full":
            g_sb = const.tile([P, OH], f32)
            b_sb = const.tile([P, OH], f32)
            nc.sync.dma_start(
                g_sb, bass.AP(tensor=gam.tensor, offset=gam.offset, ap=[[0, P], *gam.ap])
            )
            nc.sync.dma_start(
                b_sb, bass.AP(tensor=bet.tensor, offset=bet.offset, ap=[[0, P], *bet.ap])
            )

        for s in range(N_SLABS):
            xt_sl = xin.tile([P, 2, SLAB * P], f32r)
            nc.sync.dma_start(xt_sl, xt_v[:, s])
            yr_sl = yrp.tile([P, SLAB, OH], f32)
            sum_sl = stp.tile([P, SLAB], f32)
            ssq_sl = stp.tile([P, SLAB], f32)
            mv_sl = stp.tile([P, SLAB, 2], f32)
            negmu_sl = stp.tile([P, SLAB], f32)
            musq_sl = stp.tile([P, SLAB], f32)
            var_sl = stp.tile([P, SLAB], f32)
            rs_sl = stp.tile([P, SLAB], f32)
            nm_sl = stp.tile([P, SLAB], f32)
            out_a = outp.tile([P, SLAB // 2, OH], f32, tag="outh")
            out_b = outp.tile([P, SLAB // 2, OH], f32, tag="outh")

            for b in range(SLAB):
                py = psy.tile([P, OH], f32)
                nc.tensor.matmul(
                    py, xt_sl[:, 0, b * P : (b + 1) * P], w_sb[:, 0],
                    start=True, stop=False,
                )
                nc.tensor.matmul(
                    py, xt_sl[:, 1, b * P : (b + 1) * P], w_sb[:, 1],
                    start=False, stop=True,
                )
                if STATS_MODE == "accum":
                    nc.scalar.activation(
                        yr_sl[:, b], py, AF.Relu, accum_out=sum_sl[:, b : b + 1]
                    )
                    sq = sqp.tile([P, 1], f32)
                    nc.vector.tensor_tensor_reduce(
                        out=sq.broadcast_to(yr_sl[:, b].shape),
                        in0=yr_sl[:, b],
                        in1=yr_sl[:, b],
                        scale=1.0,
                        scalar=0.0,
                        op0=OP.mult,
                        op1=OP.add,
                        accum_out=ssq_sl[:, b : b + 1],
                    )
                elif STATS_MODE == "bnacc":
                    # bn stats path but with accum_out on the relu (bisect probe)
                    nc.scalar.activation(
                        yr_sl[:, b], py, AF.Relu, accum_out=sum_sl[:, b : b + 1]
                    )
                    st = sqp.tile([P, 6], f32, tag="bnst")
                    nc.vector.bn_stats(st, yr_sl[:, b])
                    nc.vector.bn_aggr(mv_sl[:, b], st)
                else:
                    nc.scalar.activation(yr_sl[:, b], py, AF.Relu)
                    st = sqp.tile([P, 6], f32, tag="bnst")
                    nc.vector.bn_stats(st, yr_sl[:, b])
                    nc.vector.bn_aggr(mv_sl[:, b], st)

            # Per-slab LN scalar math on [P, SLAB] tiles:
            #   mu = sum/OH; var = ssq/OH - mu^2; rstd = 1/sqrt(var+eps)
            #   out = y*rstd + (-mu*rstd)    (gamma_u/beta_u folded in)
            if STATS_MODE == "accum":
                nc.vector.tensor_scalar_mul(negmu_sl, sum_sl, -1.0 / OH)
                nc.vector.tensor_tensor(musq_sl, negmu_sl, negmu_sl, OP.mult)
                nc.scalar.mul(var_sl, ssq_sl, 1.0 / OH)
                nc.vector.tensor_tensor(var_sl, var_sl, musq_sl, OP.subtract)
            else:
                nc.vector.tensor_scalar_mul(negmu_sl, mv_sl[:, :, 0], -1.0)
                nc.vector.tensor_copy(var_sl, mv_sl[:, :, 1])
                if STATS_MODE == "bnacc":
                    # consume sum_sl so it isn't dead code
                    nc.vector.tensor_scalar_mul(musq_sl, sum_sl, 0.0)
            nc.scalar.activation(rs_sl, var_sl, AF.Sqrt, bias=eps_sb)
            nc.vector.reciprocal(rs_sl, rs_sl)
            nc.vector.tensor_tensor(nm_sl, negmu_sl, rs_sl, OP.mult)
            if affine_mode == "none" and (g_u != 1.0 or b_u != 0.0):
                if g_u != 1.0:
                    nc.vector.tensor_scalar_mul(rs_sl, rs_sl, float(g_u))
                nc.vector.tensor_scalar(
                    nm_sl, nm_sl, float(g_u), float(b_u), OP.mult, OP.add
                )

            for b in range(SLAB):
                rs_ap = rs_sl[:, b : b + 1]
                nm_ap = nm_sl[:, b : b + 1]
                eng = APPLY_ENGINES[b]
                ot = out_a if b < SLAB // 2 else out_b
                ob = ot[:, b % (SLAB // 2)]
                if eng == "A":
                    nc.scalar.activation(
                        ob, yr_sl[:, b], AF.Identity, bias=nm_ap, scale=rs_ap
                    )
                elif eng == "V":
                    nc.vector.tensor_scalar(
                        ob, yr_sl[:, b], rs_ap, nm_ap, OP.mult, OP.add
                    )
                else:
                    nc.gpsimd.tensor_scalar(
                        ob, yr_sl[:, b], rs_ap, nm_ap, OP.mult, OP.add
                    )
                if affine_mode == "full":
                    nc.vector.tensor_tensor(ob, ob, g_sb, OP.mult)
                    nc.gpsimd.tensor_tensor(ob, ob, b_sb, OP.add)
                if b == SLAB // 2 - 1:
                    nc.sync.dma_start(o_v[s, :, : SLAB // 2], out_a)
            nc.sync.dma_start(o_v[s, :, SLAB // 2 :], out_b)
    nc.finalize()
    return nc


def _get_prog(affine_mode, g_u, b_u):
    key = (affine_mode, g_u, b_u)
    if key not in _prog_cache:
        _prog_cache[key] = _build(affine_mode, g_u, b_u)
    return _prog_cache[key]


def _prepare(x, W_q, W_k, W_v, W_r, mix, gamma, beta):
    x = np.asarray(x, dtype=np.float32)
    W_v = np.asarray(W_v, dtype=np.float32)
    W_r = np.asarray(W_r, dtype=np.float32)
    gamma = np.asarray(gamma, dtype=np.float32)
    beta = np.asarray(beta, dtype=np.float32)
    m = 1.0 / (1.0 + np.exp(-float(np.asarray(mix).reshape(-1)[0])))
    wc = np.ascontiguousarray((m * W_v + (1.0 - m) * W_r).astype(np.float32))

    if np.all(gamma == gamma.flat[0]) and np.all(beta == beta.flat[0]):
        affine_mode, g_u, b_u = "none", float(gamma.flat[0]), float(beta.flat[0])
    else:
        affine_mode, g_u, b_u = "full", 1.0, 0.0

    x_flat = x.reshape(R * F, D_IN)
    in_maps = []
    for c in range(N_CORES):
        shard = x_flat[c * ROWS_PER_CORE : (c + 1) * ROWS_PER_CORE]
        # [p, s, ko, r] layout: contiguous 8KB per (partition, slab)
        xt_h = np.ascontiguousarray(
            shard.reshape(N_SLABS, SLAB * P, 2, P).transpose(3, 0, 2, 1)
        )
        im = {"xt": xt_h, "w": wc}
        if affine_mode == "full":
            im["gamma"] = gamma
            im["beta"] = beta
        in_maps.append(im)
    return in_maps, affine_mode, g_u, b_u


def _unpermute_out(arr):
    # [s, p, b, n] -> rows ordered (s, b, p)
    return arr.transpose(0, 2, 1, 3).reshape(ROWS_PER_CORE, OH)


def run(trace=False, **inputs):
    """Internal entry: returns (output, BassKernelResults)."""
    from concourse.bass_utils import run_bass_kernel_spmd

    in_maps, affine_mode, g_u, b_u = _prepare(**inputs)
    nc = _get_prog(affine_mode, g_u, b_u)
    res = run_bass_kernel_spmd(nc, in_maps, core_ids=list(range(N_CORES)), trace=trace)
    parts = [
        _unpermute_out(r["out"]).reshape(R // N_CORES, F, OH) for r in res.results
    ]
    return np.concatenate(parts, axis=0), res


def kernel(**inputs):
    out, _ = run(trace=False, **inputs)
    return out


# revision 29
# speedup vs baseline: 1.3116x; 1.0316x over previous
"""Trainium2 Bass kernel for nn_AttnInteractionLayer_16982300688923.

Math: the reference's einsum 'rfdh,rfoh->rfoh' contracts alpha over its own
softmax axis, which sums to exactly 1 — so the whole Q/K/softmax pipeline
collapses to out == vals.  The remaining computation is

    y   = x @ (m*W_v + (1-m)*W_r)          m = sigmoid(mix)  (host-folded)
    y   = relu(y)
    out = (y - mean(y)) * rsqrt(var(y)+eps) * gamma + beta    (LN over last dim)

Sharding: data-parallel over R across 8 cores; weights replicated.  X is
pre-transposed on the host while sharding so the contraction dim lands on
SBUF partitions with fast contiguous DMAs (fp32 has no DMA-transpose path).

Per-core device pipeline (rows = R/8 * F = 16384, fp32):
  DMA x^T slabs -> float32r matmuls accumulate Y[128,512] in PSUM ->
  ACT relu PSUM->SBUF (+sum accum) -> DVE square (+sum-of-squares accum) ->
  batched LN scalar math -> apply (y-mu)*rstd split across GPSIMD/ACT/DVE ->
  DMA out.
Uniform gamma/beta fold into the per-row scalars; per-feature gamma/beta get
two extra broadcast passes (general path).
"""

import numpy as np

R, F, D_IN = 2048, 64, 256
OH = 512  # output_dim * num_head
N_CORES = 8
ROWS_PER_CORE = (R // N_CORES) * F  # 16384
P = 128
BLOCKS = ROWS_PER_CORE // P  # 128
SLAB = 8  # 128-row blocks per slab
N_SLABS = BLOCKS // SLAB  # 16
EPS = 1e-5

# apply-pass engine per block-in-slab: balance GPSIMD / ACT / DVE
import os as _os

APPLY_ENGINES = list(_os.environ.get("K_APPLY", "GGGGGGGG"))
STATS_MODE = _os.environ.get("K_STATS", "bn")  # accum | bnacc | bn

_prog_cache = {}


def _build(affine_mode, g_u, b_u):
    """affine_mode: 'none' (uniform gamma/beta folded into scalars g_u/b_u)
    or 'full' (per-feature gamma/beta tensors applied on device)."""
    from contextlib import ExitStack

    import concourse.bass as bass
    import concourse.mybir as mybir
    import concourse.tile as tile
    from concourse import bacc

    f32 = mybir.dt.float32
    f32r = mybir.dt.float32r
    AF = mybir.ActivationFunctionType
    OP = mybir.AluOpType

    nc = bacc.Bacc(trn_type="TRN2", target_bir_lowering=False)
    # Host-permuted input: [p, s, ko, r] so each partition reads one
    # contiguous 8KB run per slab.
    xt = nc.dram_tensor(
        "xt", [P, N_SLABS, 2, SLAB * P], f32, kind="ExternalInput"
    )
    wc = nc.dram_tensor("w", [D_IN, OH], f32, kind="ExternalInput")
    if affine_mode == "full":
        gam = nc.dram_tensor("gamma", [OH], f32, kind="ExternalInput")
        bet = nc.dram_tensor("beta", [OH], f32, kind="ExternalInput")
    # Host-unpermuted output: [s, p, b, n] so each partition writes one
    # contiguous 16KB run per slab (8KB per half-slab DMA).
    out = nc.dram_tensor(
        "out", [N_SLABS, P, SLAB, OH], f32, kind="ExternalOutput"
    )

    xt_v = xt.bitcast(f32r)
    o_v = out

    with ExitStack() as ctx:
        tc = ctx.enter_context(tile.TileContext(nc))
        const = ctx.enter_context(tc.tile_pool(name="const", bufs=1))
        xin = ctx.enter_context(tc.tile_pool(name="xin", bufs=4))
        psy = ctx.enter_context(tc.tile_pool(name="psy", bufs=6, space="PSUM"))
        yrp = ctx.enter_context(tc.tile_pool(name="yrp", bufs=3))
        sqp = ctx.enter_context(tc.tile_pool(name="sqp", bufs=8))
        stp = ctx.enter_context(tc.tile_pool(name="stp", bufs=8))
        outp = ctx.enter_context(tc.tile_pool(name="outp", bufs=12))

        w_sb = const.tile([P, 2, OH], f32r)
        nc.sync.dma_start(w_sb, wc.rearrange("(ko p) n -> p ko n", p=P).bitcast(f32r))
        eps_sb = const.tile([P, 1], f32)
        nc.vector.memset(eps_sb, EPS)
        if affine_mode == "full":
            g_sb = const.tile([P, OH], f32)
            b_sb = const.tile([P, OH], f32)
            nc.sync.dma_start(
                g_sb, bass.AP(tensor=gam.tensor, offset=gam.offset, ap=[[0, P], *gam.ap])
            )
            nc.sync.dma_start(
                b_sb, bass.AP(tensor=bet.tensor, offset=bet.offset, ap=[[0, P], *bet.ap])
            )

        H = SLAB // 2  # blocks per half-slab stats group
        for s in range(N_SLABS):
            xt_sl = xin.tile([P, 2, SLAB * P], f32r)
            # split by ko: two 512KB DMAs with 4KB-contiguous runs
            nc.sync.dma_start(xt_sl[:, 0], xt_v[:, s, 0])
            nc.sync.dma_start(xt_sl[:, 1], xt_v[:, s, 1])
            yr_sl = yrp.tile([P, SLAB, OH], f32)

            for h in range(2):
                mv_h = stp.tile([P, H, 2], f32, tag="mv")
                negmu_h = stp.tile([P, H], f32, tag="negmu")
                rs_h = stp.tile([P, H], f32, tag="rs")
                nm_h = stp.tile([P, H], f32, tag="nm")
                for j in range(H):
                    b = h * H + j
                    py = psy.tile([P, OH], f32)
                    nc.tensor.matmul(
                        py, xt_sl[:, 0, b * P : (b + 1) * P], w_sb[:, 0],
                        start=True, stop=False,
                    )
                    nc.tensor.matmul(
                        py, xt_sl[:, 1, b * P : (b + 1) * P], w_sb[:, 1],
                        start=False, stop=True,
                    )
                    nc.scalar.activation(yr_sl[:, b], py, AF.Relu)
                    st = sqp.tile([P, 6], f32, tag="bnst")
                    nc.vector.bn_stats(st, yr_sl[:, b])
                    nc.vector.bn_aggr(mv_h[:, j], st)

                # Half-slab LN scalar math on [P, H] tiles:
                #   rstd = 1/sqrt(var+eps) (*g_u); nm = -mu*rstd (*g_u + b_u)
                nc.scalar.activation(rs_h, mv_h[:, :, 1], AF.Sqrt, bias=eps_sb)
                nc.vector.reciprocal(rs_h, rs_h)
                nc.vector.tensor_scalar_mul(negmu_h, mv_h[:, :, 0], -1.0)
                nc.vector.tensor_tensor(nm_h, negmu_h, rs_h, OP.mult)
                if affine_mode == "none" and (g_u != 1.0 or b_u != 0.0):
                    if g_u != 1.0:
                        nc.vector.tensor_scalar_mul(rs_h, rs_h, float(g_u))
                    nc.vector.tensor_scalar(
                        nm_h, nm_h, float(g_u), float(b_u), OP.mult, OP.add
                    )

                for q in range(2):
                    oq = outp.tile([P, 2, OH], f32, tag="outq")
                    for j2 in range(2):
                        j = q * 2 + j2
                        b = h * H + j
                        rs_ap = rs_h[:, j : j + 1]
                        nm_ap = nm_h[:, j : j + 1]
                        eng = APPLY_ENGINES[b]
                        ob = oq[:, j2]
                        if eng == "A":
                            nc.scalar.activation(
                                ob, yr_sl[:, b], AF.Identity,
                                bias=nm_ap, scale=rs_ap,
                            )
                        elif eng == "V":
                            nc.vector.tensor_scalar(
                                ob, yr_sl[:, b], rs_ap, nm_ap, OP.mult, OP.add
                            )
                        else:
                            nc.gpsimd.tensor_scalar(
                                ob, yr_sl[:, b], rs_ap, nm_ap, OP.mult, OP.add
                            )
                        if affine_mode == "full":
                            nc.vector.tensor_tensor(ob, ob, g_sb, OP.mult)
                            nc.gpsimd.tensor_tensor(ob, ob, b_sb, OP.add)
                    b0 = h * H + q * 2
                    nc.sync.dma_start(o_v[s, :, b0 : b0 + 2], oq)
    nc.finalize()
    return nc


def _get_prog(affine_mode, g_u, b_u):
    key = (affine_mode, g_u, b_u)
    if key not in _prog_cache:
        _prog_cache[key] = _build(affine_mode, g_u, b_u)
    return _prog_cache[key]


def _prepare(x, W_q, W_k, W_v, W_r, mix, gamma, beta):
    x = np.asarray(x, dtype=np.float32)
    W_v = np.asarray(W_v, dtype=np.float32)
    W_r = np.asarray(W_r, dtype=np.float32)
    gamma = np.asarray(gamma, dtype=np.float32)
    beta = np.asarray(beta, dtype=np.float32)
    m = 1.0 / (1.0 + np.exp(-float(np.asarray(mix).reshape(-1)[0])))
    wc = np.ascontiguousarray((m * W_v + (1.0 - m) * W_r).astype(np.float32))

    if np.all(gamma == gamma.flat[0]) and np.all(beta == beta.flat[0]):
        affine_mode, g_u, b_u = "none", float(gamma.flat[0]), float(beta.flat[0])
    else:
        affine_mode, g_u, b_u = "full", 1.0, 0.0

    x_flat = x.reshape(R * F, D_IN)
    in_maps = []
    for c in range(N_CORES):
        shard = x_flat[c * ROWS_PER_CORE : (c + 1) * ROWS_PER_CORE]
        # [p, s, ko, r] layout: contiguous 8KB per (partition, slab)
        xt_h = np.ascontiguousarray(
            shard.reshape(N_SLABS, SLAB * P, 2, P).transpose(3, 0, 2, 1)
        )
        im = {"xt": xt_h, "w": wc}
        if affine_mode == "full":
            im["gamma"] = gamma
            im["beta"] = beta
        in_maps.append(im)
    return in_maps, affine_mode, g_u, b_u


def _unpermute_out(arr):
    # [s, p, b, n] -> rows ordered (s, b, p)
    return arr.transpose(0, 2, 1, 3).reshape(ROWS_PER_CORE, OH)


def run(trace=False, **inputs):
    """Internal entry: returns (output, BassKernelResults)."""
    from concourse.bass_utils import run_bass_kernel_spmd

    in_maps, affine_mode, g_u, b_u = _prepare(**inputs)
    nc = _get_prog(affine_mode, g_u, b_u)
    res = run_bass_kernel_spmd(nc, in_maps, core_ids=list(range(N_CORES)), trace=trace)
    parts = [
        _unpermute_out(r["out"]).reshape(R // N_CORES, F, OH) for r in res.results
    ]
    return np.concatenate(parts, axis=0), res


def kernel(**inputs):
    out, _ = run(trace=False, **inputs)
    return out


# revision 30
# speedup vs baseline: 1.3598x; 1.0367x over previous
"""Trainium2 Bass kernel for nn_AttnInteractionLayer_16982300688923.

Math: the reference's einsum 'rfdh,rfoh->rfoh' contracts alpha over its own
softmax axis, which sums to exactly 1 — so the whole Q/K/softmax pipeline
collapses to out == vals.  The remaining computation is

    y   = x @ (m*W_v + (1-m)*W_r)          m = sigmoid(mix)  (host-folded)
    y   = relu(y)
    out = (y - mean(y)) * rsqrt(var(y)+eps) * gamma + beta    (LN over last dim)

Sharding: data-parallel over R across 8 cores; weights replicated.  X is
pre-transposed on the host while sharding so the contraction dim lands on
SBUF partitions with fast contiguous DMAs (fp32 has no DMA-transpose path).

Per-core device pipeline (rows = R/8 * F = 16384, fp32):
  DMA x^T slabs -> float32r matmuls accumulate Y[128,512] in PSUM ->
  ACT relu PSUM->SBUF (+sum accum) -> DVE square (+sum-of-squares accum) ->
  batched LN scalar math -> apply (y-mu)*rstd split across GPSIMD/ACT/DVE ->
  DMA out.
Uniform gamma/beta fold into the per-row scalars; per-feature gamma/beta get
two extra broadcast passes (general path).
"""

import numpy as np

R, F, D_IN = 2048, 64, 256
OH = 512  # output_dim * num_head
N_CORES = 8
ROWS_PER_CORE = (R // N_CORES) * F  # 16384
P = 128
BLOCKS = ROWS_PER_CORE // P  # 128
SLAB = 8  # 128-row blocks per slab
N_SLABS = BLOCKS // SLAB  # 16
EPS = 1e-5

# apply-pass engine per block-in-slab: balance GPSIMD / ACT / DVE
import os as _os

APPLY_ENGINES = list(_os.environ.get("K_APPLY", "GGGGGGGG"))
STATS_MODE = _os.environ.get("K_STATS", "bn")  # accum | bnacc | bn

_prog_cache = {}


def _build(affine_mode, g_u, b_u):
    """affine_mode: 'none' (uniform gamma/beta folded into scalars g_u/b_u)
    or 'full' (per-feature gamma/beta tensors applied on device)."""
    from contextlib import ExitStack

    import concourse.bass as bass
    import concourse.mybir as mybir
    import concourse.tile as tile
    from concourse import bacc

    f32 = mybir.dt.float32
    f32r = mybir.dt.float32r
    AF = mybir.ActivationFunctionType
    OP = mybir.AluOpType

    nc = bacc.Bacc(trn_type="TRN2", target_bir_lowering=False)
    # Host-permuted input: [p, s, ko, r] so each partition reads one
    # contiguous 8KB run per slab.
    xt = nc.dram_tensor(
        "xt", [P, N_SLABS, 2, SLAB * P], f32, kind="ExternalInput"
    )
    wc = nc.dram_tensor("w", [D_IN, OH], f32, kind="ExternalInput")
    if affine_mode == "full":
        gam = nc.dram_tensor("gamma", [OH], f32, kind="ExternalInput")
        bet = nc.dram_tensor("beta", [OH], f32, kind="ExternalInput")
    # Host-unpermuted output: [s, p, b, n] so each partition writes one
    # contiguous 16KB run per slab (8KB per half-slab DMA).
    out = nc.dram_tensor(
        "out", [N_SLABS, P, SLAB, OH], f32, kind="ExternalOutput"
    )

    xt_v = xt.bitcast(f32r)
    o_v = out

    with ExitStack() as ctx:
        tc = ctx.enter_context(tile.TileContext(nc))
        const = ctx.enter_context(tc.tile_pool(name="const", bufs=1))
        xin = ctx.enter_context(tc.tile_pool(name="xin", bufs=4))
        psy = ctx.enter_context(tc.tile_pool(name="psy", bufs=6, space="PSUM"))
        yrp = ctx.enter_context(tc.tile_pool(name="yrp", bufs=3))
        sqp = ctx.enter_context(tc.tile_pool(name="sqp", bufs=8))
        stp = ctx.enter_context(tc.tile_pool(name="stp", bufs=8))
        outp = ctx.enter_context(tc.tile_pool(name="outp", bufs=12))

        w_sb = const.tile([P, 2, OH], f32r)
        nc.sync.dma_start(w_sb, wc.rearrange("(ko p) n -> p ko n", p=P).bitcast(f32r))
        eps_sb = const.tile([P, 1], f32)
        nc.vector.memset(eps_sb, EPS)
        if affine_mode == "full":
            g_sb = const.tile([P, OH], f32)
            b_sb = const.tile([P, OH], f32)
            nc.sync.dma_start(
                g_sb, bass.AP(tensor=gam.tensor, offset=gam.offset, ap=[[0, P], *gam.ap])
            )
            nc.sync.dma_start(
                b_sb, bass.AP(tensor=bet.tensor, offset=bet.offset, ap=[[0, P], *bet.ap])
            )

        H = SLAB // 2  # blocks per half-slab stats group
        for s in range(N_SLABS):
            xt_sl = xin.tile([P, 2, SLAB * P], f32r)
            # input rides the Scalar HWDGE queue so it never sits behind the
            # output backlog in the Sync queue's per-engine FIFOs
            nc.scalar.dma_start(xt_sl, xt_v[:, s])
            yr_sl = yrp.tile([P, SLAB, OH], f32)

            for h in range(2):
                mv_h = stp.tile([P, H, 2], f32, tag="mv")
                negmu_h = stp.tile([P, H], f32, tag="negmu")
                rs_h = stp.tile([P, H], f32, tag="rs")
                nm_h = stp.tile([P, H], f32, tag="nm")
                for j in range(H):
                    b = h * H + j
                    py = psy.tile([P, OH], f32)
                    nc.tensor.matmul(
                        py, xt_sl[:, 0, b * P : (b + 1) * P], w_sb[:, 0],
                        start=True, stop=False,
                    )
                    nc.tensor.matmul(
                        py, xt_sl[:, 1, b * P : (b + 1) * P], w_sb[:, 1],
                        start=False, stop=True,
                    )
                    nc.scalar.activation(yr_sl[:, b], py, AF.Relu)
                    st = sqp.tile([P, 6], f32, tag="bnst")
                    nc.vector.bn_stats(st, yr_sl[:, b])
                    nc.vector.bn_aggr(mv_h[:, j], st)

                # Half-slab LN scalar math on [P, H] tiles:
                #   rstd = 1/sqrt(var+eps) (*g_u); nm = -mu*rstd (*g_u + b_u)
                nc.scalar.activation(rs_h, mv_h[:, :, 1], AF.Sqrt, bias=eps_sb)
                nc.vector.reciprocal(rs_h, rs_h)
                nc.vector.tensor_scalar_mul(negmu_h, mv_h[:, :, 0], -1.0)
                nc.vector.tensor_tensor(nm_h, negmu_h, rs_h, OP.mult)
                if affine_mode == "none" and (g_u != 1.0 or b_u != 0.0):
                    if g_u != 1.0:
                        nc.vector.tensor_scalar_mul(rs_h, rs_h, float(g_u))
                    nc.vector.tensor_scalar(
                        nm_h, nm_h, float(g_u), float(b_u), OP.mult, OP.add
                    )

                for q in range(2):
                    oq = outp.tile([P, 2, OH], f32, tag="outq")
                    for j2 in range(2):
                        j = q * 2 + j2
                        b = h * H + j
                        rs_ap = rs_h[:, j : j + 1]
                        nm_ap = nm_h[:, j : j + 1]
                        eng = APPLY_ENGINES[b]
                        ob = oq[:, j2]
                        if eng == "A":
                            nc.scalar.activation(
                                ob, yr_sl[:, b], AF.Identity,
                                bias=nm_ap, scale=rs_ap,
                            )
                        elif eng == "V":
                            nc.vector.tensor_scalar(
                                ob, yr_sl[:, b], rs_ap, nm_ap, OP.mult, OP.add
                            )
                        else:
                            nc.gpsimd.tensor_scalar(
                                ob, yr_sl[:, b], rs_ap, nm_ap, OP.mult, OP.add
                            )
                        if affine_mode == "full":
                            nc.vector.tensor_tensor(ob, ob, g_sb, OP.mult)
                            nc.gpsimd.tensor_tensor(ob, ob, b_sb, OP.add)
                    b0 = h * H + q * 2
                    nc.sync.dma_start(o_v[s, :, b0 : b0 + 2], oq)
    nc.finalize()
    return nc


def _get_prog(affine_mode, g_u, b_u):
    key = (affine_mode, g_u, b_u)
    if key not in _prog_cache:
        _prog_cache[key] = _build(affine_mode, g_u, b_u)
    return _prog_cache[key]


def _prepare(x, W_q, W_k, W_v, W_r, mix, gamma, beta):
    x = np.asarray(x, dtype=np.float32)
    W_v = np.asarray(W_v, dtype=np.float32)
    W_r = np.asarray(W_r, dtype=np.float32)
    gamma = np.asarray(gamma, dtype=np.float32)
    beta = np.asarray(beta, dtype=np.float32)
    m = 1.0 / (1.0 + np.exp(-float(np.asarray(mix).reshape(-1)[0])))
    wc = np.ascontiguousarray((m * W_v + (1.0 - m) * W_r).astype(np.float32))

    if np.all(gamma == gamma.flat[0]) and np.all(beta == beta.flat[0]):
        affine_mode, g_u, b_u = "none", float(gamma.flat[0]), float(beta.flat[0])
    else:
        affine_mode, g_u, b_u = "full", 1.0, 0.0

    x_flat = x.reshape(R * F, D_IN)
    in_maps = []
    for c in range(N_CORES):
        shard = x_flat[c * ROWS_PER_CORE : (c + 1) * ROWS_PER_CORE]
        # [p, s, ko, r] layout: contiguous 8KB per (partition, slab)
        xt_h = np.ascontiguousarray(
            shard.reshape(N_SLABS, SLAB * P, 2, P).transpose(3, 0, 2, 1)
        )
        im = {"xt": xt_h, "w": wc}
        if affine_mode == "full":
            im["gamma"] = gamma
            im["beta"] = beta
        in_maps.append(im)
    return in_maps, affine_mode, g_u, b_u


def _unpermute_out(arr):
    # [s, p, b, n] -> rows ordered (s, b, p)
    return arr.transpose(0, 2, 1, 3).reshape(ROWS_PER_CORE, OH)


def run(trace=False, **inputs):
    """Internal entry: returns (output, BassKernelResults)."""
    from concourse.bass_utils import run_bass_kernel_spmd

    in_maps, affine_mode, g_u, b_u = _prepare(**inputs)
    nc = _get_prog(affine_mode, g_u, b_u)
    res = run_bass_kernel_spmd(nc, in_maps, core_ids=list(range(N_CORES)), trace=trace)
    parts = [
        _unpermute_out(r["out"]).reshape(R // N_CORES, F, OH) for r in res.results
    ]
    return np.concatenate(parts, axis=0), res


def kernel(**inputs):
    out, _ = run(trace=False, **inputs)
    return out
